# revision 46
# baseline (speedup 1.0000x reference)
"""MiniSTU Trainium2 kernel.

Reformulation (no FFT): per batch b,
    out = T @ (x @ Mp) + sgn ⊙ (T @ (sgn ⊙ (x @ Mm)))
where T is the lower-triangular block-Toeplitz matrix from phi and
sgn[l] = (-1)^l.  Polyphase split: with B_even = A+ + A-, B_odd = A+ - A-
(A+ = x@Mp, A- = sgn⊙(x@Mm)), even output rows need only (T@B_even)_even
and odd rows only (T@B_odd)_odd — half the convolution MACs.  The two
M=64 matmuls per Toeplitz block run concurrently in disjoint PE
column-groups via tile_position (0,0)/(0,64).

8 cores = batch(2) x output-quarter(4), no collectives; fp16 operands,
fp32 PSUM accumulation; two k-halves to fit SBUF.
"""

import numpy as np

B, L, D, O, K, P = 2, 2048, 512, 512, 16, 128
NB = L // P       # 16 l-blocks
KH = 2            # k halves
KPH = K // KH     # 8 filters per half
NOQ = 4           # o-quarters
OS = O // NOQ     # 128 per-core o slice
N_CORES = 8

_cache = {}


def _build_bass(reps=1):
    import contextlib
    import concourse.mybir as mybir
    import concourse.tile as tile
    from concourse import bacc

    dt = mybir.dt
    f16, f32 = dt.float16, dt.float32

    nc = bacc.Bacc("TRN2", target_bir_lowering=False, debug=False,
                   num_devices=N_CORES)

    xt_d = nc.dram_tensor("xt", [P, 4, L], f16, kind="ExternalInput")
    mx_d = nc.dram_tensor("mx", [P, 4, K * 2 * OS], f16, kind="ExternalInput")
    ph_d = nc.dram_tensor("ph", [KH, 4, P, 4 * KPH * P], f16, kind="ExternalInput")
    out_d = nc.dram_tensor("out", [P, NB * OS], f32, kind="ExternalOutput")

    CH = KPH * 2 * OS          # 2048 columns per k-half in mx/a
    with tile.TileContext(nc) as tc:
        with (
            tc.tile_pool(name="const", bufs=1) as cpool,
            tc.tile_pool(name="phpool", bufs=1) as phpool,
            tc.tile_pool(name="apool", bufs=1) as apool,
            tc.tile_pool(name="opool", bufs=1) as opool,
        ):
            xt = cpool.tile([P, 4, L], f16, tag="xt")
            mx = cpool.tile([P, 4, K * 2 * OS], f16, tag="mx")
            a_sb = apool.tile([P, NB, CH], f16, tag="a")
            outacc = opool.tile([P, NB, OS], f32, tag="outacc")

            for dc in range(4):
                nc.sync.dma_start(out=xt[:, dc, :], in_=xt_d[:, dc, :])
                nc.sync.dma_start(out=mx[:, dc, :], in_=mx_d[:, dc, :])

            loop_cm = (tc.For_i(0, reps, 1,
                                hint_engines=(mybir.EngineType.PE,
                                              mybir.EngineType.DVE))
                       if reps > 1 else contextlib.nullcontext())
            with loop_cm:
                _emit_body(nc, tc, mybir, f16, f32, xt, mx, ph_d, phpool,
                           a_sb, outacc, out_d)

    nc.compile()
    return nc


def _emit_body(nc, tc, mybir, f16, f32, xt, mx, ph_d, phpool,
               a_sb, outacc, out_d):
    CH = KPH * 2 * OS
    # even rows of out_d / odd rows, as [64, NB*OS] strided views
    od_even = out_d[:].rearrange("(h two) c -> two h c", two=2)[0]
    od_odd = out_d[:].rearrange("(h two) c -> two h c", two=2)[1]

    for kh in range(KH):
        ph = phpool.tile([P, NB * KPH * P], f16, tag="ph")
        for q in range(4):
            nc.sync.dma_start(out=ph[:, q * 4096:(q + 1) * 4096],
                              in_=ph_d[kh, q])

        # ---- stage 1: psum = [x@Mp | x@Mm] per l-tile, drained to
        # B_even = plus + sgn*minus, B_odd = plus - sgn*minus (fp16)
        with tc.tile_pool(name="ps1", bufs=2, space="PSUM") as ps1pool:
            for lt in range(NB):
                ps = ps1pool.tile([P, 2048], f32, tag="ps1")
                for dc in range(4):
                    for n in range(4):
                        c0 = kh * CH + n * 512
                        nc.tensor.matmul(
                            ps[:, n * 512:(n + 1) * 512],
                            xt[:, dc, lt * P:(lt + 1) * P],
                            mx[:, dc, c0:c0 + 512],
                            start=(dc == 0), stop=(dc == 3),
                        )
                psv = ps[:].rearrange("p (a s o) -> p a s o", a=KPH, s=2, o=OS)
                av = a_sb[:, lt, :].rearrange("p (a g o) -> p a g o",
                                              a=KPH, g=2, o=OS)
                nc.vector.tensor_copy(av[0:64, :, 0, :], psv[0:64, :, 0, :])
                nc.vector.tensor_copy(av[64:128, :, 0, :], psv[64:128, :, 1, :])
                nc.vector.tensor_copy(av[0:64, :, 1, :], psv[0:64, :, 1, :])
                nc.vector.tensor_copy(av[64:128, :, 1, :], psv[64:128, :, 0, :])

        # ---- stage 2: even/odd polyphase conv, (d,kl)-outer, M=64
        # col-tiled pairs.  4 accumulators x 2 parities = 8 PSUM banks.
        with tc.tile_pool(name="ps2", bufs=1, space="PSUM") as ps2pool:
            for ig in range(4):
                i_lo = ig * 4
                ps2e = ps2pool.tile([P, 4, 512], f32, tag="ps2e")
                ps2o = ps2pool.tile([P, 4, 512], f32, tag="ps2o")
                for d in range(NB):
                    j_lo = max(0, i_lo - d)
                    j_hi = min(NB, i_lo + 4 - d)
                    if j_hi <= j_lo:
                        continue
                    for kl in range(KPH):
                        blk = (d * KPH + kl) * P
                        for J in range(j_lo, j_hi):
                            I = J + d
                            st = (d == 0 and kl == 0)
                            sp = (d == I and kl == KPH - 1)
                            nc.tensor.matmul(
                                ps2e[0:64, I - i_lo, 0:OS],
                                ph[:, blk:blk + 64],
                                a_sb[:, J, kl * 2 * OS:kl * 2 * OS + OS],
                                start=st, stop=sp, tile_position=(0, 0),
                            )
                            nc.tensor.matmul(
                                ps2o[64:128, I - i_lo, 0:OS],
                                ph[:, blk + 64:blk + P],
                                a_sb[:, J, kl * 2 * OS + OS:(kl + 1) * 2 * OS],
                                start=st, stop=sp, tile_position=(0, 64),
                            )
                    if i_lo <= d < i_lo + 4:
                        I = d
                        if kh == 0:
                            nc.vector.tensor_copy(
                                outacc[0:64, I, :], ps2e[0:64, I - i_lo, 0:OS])
                            nc.vector.tensor_copy(
                                outacc[64:128, I, :], ps2o[64:128, I - i_lo, 0:OS])
                        else:
                            nc.vector.tensor_add(
                                outacc[0:64, I, :], outacc[0:64, I, :],
                                ps2e[0:64, I - i_lo, 0:OS])
                            nc.vector.tensor_add(
                                outacc[64:128, I, :], outacc[64:128, I, :],
                                ps2o[64:128, I - i_lo, 0:OS])
                            nc.sync.dma_start(
                                out=od_even[:, I * OS:(I + 1) * OS],
                                in_=outacc[0:64, I, :])
                            nc.sync.dma_start(
                                out=od_odd[:, I * OS:(I + 1) * OS],
                                in_=outacc[64:128, I, :])


def _prep_inputs(x, phi, M_phi_plus, M_phi_minus):
    """Host-side shard prep. Returns list of 8 input dicts (cores = b*4 + oq).

    All sign handling is done here: s=0 carries Msum=Mp+Mm, s=1 Mdif=Mp-Mm,
    and l-rows are parity-permuted (even rows first within each 128-block),
    so B_even/B_odd on device are plain partition-range copies."""
    perm = np.concatenate([2 * np.arange(64), 2 * np.arange(64) + 1])  # [128]

    # xt[p, dc, lt*128 + q] = x[b, lt*128 + perm[q], dc*128+p]
    xts = []
    for b in range(B):
        xb = x[b].reshape(NB, P, D)[:, perm, :].reshape(L, D)
        xt = np.ascontiguousarray(
            xb.T.reshape(4, P, L).transpose(1, 0, 2)).astype(np.float16)
        xts.append(xt)

    # mx[p, dc, k*256 + s*128 + oo] = M_s[k, dc*128+p, oq*128+oo]
    mcat = np.stack([M_phi_plus + M_phi_minus,
                     M_phi_plus - M_phi_minus], axis=1)  # [K, 2, D, O]
    mxs = []
    for oq in range(NOQ):
        m = mcat[:, :, :, oq * OS:(oq + 1) * OS]        # [K, 2, D, OS]
        m = m.transpose(2, 0, 1, 3).reshape(D, K * 2 * OS)
        mx = np.ascontiguousarray(
            m.reshape(4, P, K * 2 * OS).transpose(1, 0, 2)).astype(np.float16)
        mxs.append(mx)

    # parity-split Toeplitz blocks: for block (d, k), column m of the
    # even half is output row 2m, of the odd half row 2m+1:
    #   ph[.., (dq, kl, par, m)] = phi[d*P + (2m+par) - pp, kh*KPH+kl]
    # contraction rows (pp) use the same parity permutation as xt's l-rows
    pcol = np.concatenate([2 * np.arange(64), 2 * np.arange(64) + 1])  # [128]
    diff = pcol[None, :] - pcol[:, None]                # [pp', m'] = p - pp
    v = np.arange(NB)[:, None, None] * P + diff[None]   # [d, pp, m']
    valid = v >= 0
    phb = np.zeros((NB, P, P, K), dtype=np.float32)     # [d, pp, m', k]
    phb[valid] = phi[v[valid], :]
    # [d, pp, m', (kh, kl)] -> [kh, q, pp, dq, kl, m']
    phb = phb.reshape(4, 4, P, P, KH, KPH).transpose(4, 0, 2, 1, 5, 3)
    ph = np.ascontiguousarray(phb.reshape(KH, 4, P, 4 * KPH * P)).astype(np.float16)

    in_maps = []
    for b in range(B):
        for oq in range(NOQ):
            in_maps.append({"xt": xts[b], "mx": mxs[oq], "ph": ph})
    return in_maps


def kernel(x, phi, M_phi_plus, M_phi_minus):
    from concourse.bass_utils import run_bass_kernel_spmd

    x = np.asarray(x, dtype=np.float32)
    phi = np.asarray(phi, dtype=np.float32)
    M_phi_plus = np.asarray(M_phi_plus, dtype=np.float32)
    M_phi_minus = np.asarray(M_phi_minus, dtype=np.float32)

    if "nc" not in _cache:
        _cache["nc"] = _build_bass()
    nc = _cache["nc"]

    in_maps = _prep_inputs(x, phi, M_phi_plus, M_phi_minus)
    results = run_bass_kernel_spmd(nc, in_maps, core_ids=list(range(N_CORES)))

    out = np.empty((B, L, O), dtype=np.float32)
    for c in range(N_CORES):
        b, oq = divmod(c, NOQ)
        r = results.results[c]["out"]                   # [P, NB*OS]
        blk = r.reshape(P, NB, OS).transpose(1, 0, 2).reshape(L, OS)
        out[b, :, oq * OS:(oq + 1) * OS] = blk
    return out


# revision 48
# speedup vs baseline: 1.8017x; 1.8017x over previous
"""MiniSTU Trainium2 kernel.

Reformulation (no FFT): per batch b,
    out = T @ (x @ Mp) + sgn ⊙ (T @ (sgn ⊙ (x @ Mm)))
where T is the lower-triangular block-Toeplitz matrix from phi and
sgn[l] = (-1)^l.  Polyphase split: with B_even = A+ + A-, B_odd = A+ - A-
(A+ = x@Mp, A- = sgn⊙(x@Mm)), even output rows need only (T@B_even)_even
and odd rows only (T@B_odd)_odd — half the convolution MACs.  The two
M=64 matmuls per Toeplitz block run concurrently in disjoint PE
column-groups via tile_position (0,0)/(0,64).

8 cores = batch(2) x output-quarter(4), no collectives; fp16 operands,
fp32 PSUM accumulation; two k-halves to fit SBUF.
"""

import numpy as np

B, L, D, O, K, P = 2, 2048, 512, 512, 16, 128
NB = L // P       # 16 l-blocks
KH = 2            # k halves
KPH = K // KH     # 8 filters per half
NOQ = 4           # o-quarters
OS = O // NOQ     # 128 per-core o slice
N_CORES = 8

_cache = {}


def _build_bass(reps=1):
    import contextlib
    import concourse.mybir as mybir
    import concourse.tile as tile
    from concourse import bacc

    dt = mybir.dt
    f16, f32 = dt.float16, dt.float32

    nc = bacc.Bacc("TRN2", target_bir_lowering=False, debug=False,
                   num_devices=N_CORES)

    xt_d = nc.dram_tensor("xt", [P, 4, L], f16, kind="ExternalInput")
    mx_d = nc.dram_tensor("mx", [P, 4, K * 2 * OS], f16, kind="ExternalInput")
    ph_d = nc.dram_tensor("ph", [KH, 4, P, 4 * KPH * P], f16, kind="ExternalInput")
    out_d = nc.dram_tensor("out", [P, NB * OS], f32, kind="ExternalOutput")

    CH = KPH * 2 * OS          # 2048 columns per k-half in mx/a
    with tile.TileContext(nc) as tc:
        with (
            tc.tile_pool(name="const", bufs=1) as cpool,
            tc.tile_pool(name="phpool", bufs=1) as phpool,
            tc.tile_pool(name="apool", bufs=1) as apool,
            tc.tile_pool(name="opool", bufs=1) as opool,
        ):
            xt = cpool.tile([P, 4, L], f16, tag="xt")
            mx = cpool.tile([P, 4, K * 2 * OS], f16, tag="mx")
            a_sb = apool.tile([P, NB, CH], f16, tag="a")
            outacc = opool.tile([P, NB, OS], f32, tag="outacc")

            for dc in range(4):
                nc.sync.dma_start(out=xt[:, dc, :], in_=xt_d[:, dc, :])
                nc.sync.dma_start(out=mx[:, dc, :], in_=mx_d[:, dc, :])

            loop_cm = (tc.For_i(0, reps, 1,
                                hint_engines=(mybir.EngineType.PE,
                                              mybir.EngineType.DVE))
                       if reps > 1 else contextlib.nullcontext())
            with loop_cm:
                _emit_body(nc, tc, mybir, f16, f32, xt, mx, ph_d, phpool,
                           a_sb, outacc, out_d)

    nc.compile()
    return nc


def _emit_body(nc, tc, mybir, f16, f32, xt, mx, ph_d, phpool,
               a_sb, outacc, out_d):
    CH = KPH * 2 * OS
    # even rows of out_d / odd rows, as [64, NB*OS] strided views
    od_even = out_d[:].rearrange("(h two) c -> two h c", two=2)[0]
    od_odd = out_d[:].rearrange("(h two) c -> two h c", two=2)[1]

    for kh in range(KH):
        ph = phpool.tile([P, NB * KPH * P], f16, tag="ph")
        for q in range(4):
            nc.sync.dma_start(out=ph[:, q * 4096:(q + 1) * 4096],
                              in_=ph_d[kh, q])

        # ---- stage 1: psum = [x@Mp | x@Mm] per l-tile, drained to
        # B_even = plus + sgn*minus, B_odd = plus - sgn*minus (fp16)
        with tc.tile_pool(name="ps1", bufs=2, space="PSUM") as ps1pool:
            for lt in range(NB):
                ps = ps1pool.tile([P, 2048], f32, tag="ps1")
                for dc in range(4):
                    for n in range(4):
                        c0 = kh * CH + n * 512
                        nc.tensor.matmul(
                            ps[:, n * 512:(n + 1) * 512],
                            xt[:, dc, lt * P:(lt + 1) * P],
                            mx[:, dc, c0:c0 + 512],
                            start=(dc == 0), stop=(dc == 3),
                        )
                psv = ps[:].rearrange("p (a s o) -> p a s o", a=KPH, s=2, o=OS)
                av = a_sb[:, lt, :].rearrange("p (a g o) -> p a g o",
                                              a=KPH, g=2, o=OS)
                nc.vector.tensor_copy(av[0:64, :, 0, :], psv[0:64, :, 0, :])
                nc.vector.tensor_copy(av[64:128, :, 0, :], psv[64:128, :, 1, :])
                nc.vector.tensor_copy(av[0:64, :, 1, :], psv[0:64, :, 1, :])
                nc.vector.tensor_copy(av[64:128, :, 1, :], psv[64:128, :, 0, :])

        # ---- stage 2: even/odd polyphase conv, (d,kl)-outer, M=64
        # col-tiled parity pairs; aligned J-runs fused into up-to-N=512
        # MMs (accumulator quad (4q..4q+3) fills one PSUM bank; the quad
        # group stops at d == 4q+3 where only I=d's contribution exists).
        # 4 quads x 2 parities = 8 banks -> single pass per k-half.
        with tc.tile_pool(name="ps2", bufs=1, space="PSUM") as ps2pool:
            if True:
                ps2e = ps2pool.tile([P, 4, 512], f32, tag="ps2e")
                ps2o = ps2pool.tile([P, 4, 512], f32, tag="ps2o")
                for d in range(NB):
                    j_hi = NB - d
                    segs = []
                    J = 0
                    while J < j_hi:
                        w = min(4 - ((J + d) % 4), j_hi - J)
                        segs.append((J, w))
                        J += w
                    for kl in range(KPH):
                        blk = (d * KPH + kl) * P
                        for (J0, w) in segs:
                            I0 = J0 + d
                            q = I0 // 4
                            off = (I0 % 4) * OS
                            st = (d == 0 and kl == 0)
                            sp = (J0 == 0 and d % 4 == 3 and kl == KPH - 1)
                            nc.tensor.matmul(
                                ps2e[0:64, q, off:off + w * OS],
                                ph[:, blk:blk + 64],
                                a_sb[:, J0:J0 + w,
                                     kl * 2 * OS:kl * 2 * OS + OS],
                                start=st, stop=sp, tile_position=(0, 0),
                            )
                            nc.tensor.matmul(
                                ps2o[64:128, q, off:off + w * OS],
                                ph[:, blk + 64:blk + P],
                                a_sb[:, J0:J0 + w,
                                     kl * 2 * OS + OS:(kl + 1) * 2 * OS],
                                start=st, stop=sp, tile_position=(0, 64),
                            )
                    # quad q = d//4 closes after d == 4q+3
                    if d % 4 == 3:
                        q = d // 4
                        Ia = 4 * q
                        pse = ps2e[0:64, q, :].rearrange(
                            "p (i o) -> p i o", i=4, o=OS)
                        pso = ps2o[64:128, q, :].rearrange(
                            "p (i o) -> p i o", i=4, o=OS)
                        if kh == 0:
                            nc.vector.tensor_copy(
                                outacc[0:64, Ia:Ia + 4, :], pse)
                            nc.vector.tensor_copy(
                                outacc[64:128, Ia:Ia + 4, :], pso)
                        else:
                            nc.vector.tensor_add(
                                outacc[0:64, Ia:Ia + 4, :],
                                outacc[0:64, Ia:Ia + 4, :], pse)
                            nc.vector.tensor_add(
                                outacc[64:128, Ia:Ia + 4, :],
                                outacc[64:128, Ia:Ia + 4, :], pso)
                            nc.sync.dma_start(
                                out=od_even[:, Ia * OS:(Ia + 4) * OS],
                                in_=outacc[0:64, Ia:Ia + 4, :])
                            nc.sync.dma_start(
                                out=od_odd[:, Ia * OS:(Ia + 4) * OS],
                                in_=outacc[64:128, Ia:Ia + 4, :])


def _prep_inputs(x, phi, M_phi_plus, M_phi_minus):
    """Host-side shard prep. Returns list of 8 input dicts (cores = b*4 + oq).

    All sign handling is done here: s=0 carries Msum=Mp+Mm, s=1 Mdif=Mp-Mm,
    and l-rows are parity-permuted (even rows first within each 128-block),
    so B_even/B_odd on device are plain partition-range copies."""
    perm = np.concatenate([2 * np.arange(64), 2 * np.arange(64) + 1])  # [128]

    # xt[p, dc, lt*128 + q] = x[b, lt*128 + perm[q], dc*128+p]
    xts = []
    for b in range(B):
        xb = x[b].reshape(NB, P, D)[:, perm, :].reshape(L, D)
        xt = np.ascontiguousarray(
            xb.T.reshape(4, P, L).transpose(1, 0, 2)).astype(np.float16)
        xts.append(xt)

    # mx[p, dc, k*256 + s*128 + oo] = M_s[k, dc*128+p, oq*128+oo]
    mcat = np.stack([M_phi_plus + M_phi_minus,
                     M_phi_plus - M_phi_minus], axis=1)  # [K, 2, D, O]
    mxs = []
    for oq in range(NOQ):
        m = mcat[:, :, :, oq * OS:(oq + 1) * OS]        # [K, 2, D, OS]
        m = m.transpose(2, 0, 1, 3).reshape(D, K * 2 * OS)
        mx = np.ascontiguousarray(
            m.reshape(4, P, K * 2 * OS).transpose(1, 0, 2)).astype(np.float16)
        mxs.append(mx)

    # parity-split Toeplitz blocks: for block (d, k), column m of the
    # even half is output row 2m, of the odd half row 2m+1:
    #   ph[.., (dq, kl, par, m)] = phi[d*P + (2m+par) - pp, kh*KPH+kl]
    # contraction rows (pp) use the same parity permutation as xt's l-rows
    pcol = np.concatenate([2 * np.arange(64), 2 * np.arange(64) + 1])  # [128]
    diff = pcol[None, :] - pcol[:, None]                # [pp', m'] = p - pp
    v = np.arange(NB)[:, None, None] * P + diff[None]   # [d, pp, m']
    valid = v >= 0
    phb = np.zeros((NB, P, P, K), dtype=np.float32)     # [d, pp, m', k]
    phb[valid] = phi[v[valid], :]
    # [d, pp, m', (kh, kl)] -> [kh, q, pp, dq, kl, m']
    phb = phb.reshape(4, 4, P, P, KH, KPH).transpose(4, 0, 2, 1, 5, 3)
    ph = np.ascontiguousarray(phb.reshape(KH, 4, P, 4 * KPH * P)).astype(np.float16)

    in_maps = []
    for b in range(B):
        for oq in range(NOQ):
            in_maps.append({"xt": xts[b], "mx": mxs[oq], "ph": ph})
    return in_maps


def kernel(x, phi, M_phi_plus, M_phi_minus):
    from concourse.bass_utils import run_bass_kernel_spmd

    x = np.asarray(x, dtype=np.float32)
    phi = np.asarray(phi, dtype=np.float32)
    M_phi_plus = np.asarray(M_phi_plus, dtype=np.float32)
    M_phi_minus = np.asarray(M_phi_minus, dtype=np.float32)

    if "nc" not in _cache:
        _cache["nc"] = _build_bass()
    nc = _cache["nc"]

    in_maps = _prep_inputs(x, phi, M_phi_plus, M_phi_minus)
    results = run_bass_kernel_spmd(nc, in_maps, core_ids=list(range(N_CORES)))

    out = np.empty((B, L, O), dtype=np.float32)
    for c in range(N_CORES):
        b, oq = divmod(c, NOQ)
        r = results.results[c]["out"]                   # [P, NB*OS]
        blk = r.reshape(P, NB, OS).transpose(1, 0, 2).reshape(L, OS)
        out[b, :, oq * OS:(oq + 1) * OS] = blk
    return out
